# revision 36
# baseline (speedup 1.0000x reference)
"""Trainium2 Bass kernel for NeighborCompressedNN (retrieval kNN + gated MLP).

Strategy (query-parallel over 8 NeuronCores, no collectives):
  - Each core owns 128 of the 1024 queries and scans the full database.
  - Selection score s[q,n] = x_q . X_n - ||X_n||^2/2 (monotonic in -dist^2
    per query), computed with f32r (TF32-class) matmuls at 1 PE cycle per
    column (4x the fp32 rate; measured end-to-end rel err ~5e-3 from
    boundary-rank swaps only — the gathered rows stay exact fp32).
  - Per 4096-column scan group (two PSUM tiles drained to one SBUF tile by
    the scalar engine): the vector engine takes top-8 values (InstMax) and
    their exact in-group positions (InstMaxIndex) — the 2-pass DVE scan is
    the hard floor on this target: GPSIMD has no HW elementwise ops, DMA
    accum supports add only, and ACT is 1-input, so the max8 work cannot
    be folded onto any other engine.
  - Final top-32 merge over the 49*8=392 candidates (4 rounds of
    max8/max_index/match_replace); per winner: index extraction via an
    is_equal+mult+accum scan, a [128]-row indirect-DMA gather of [X|y],
    a PE transpose, then the gate/tanh/sigmoid MLP head entirely on-chip
    with the neighbor-sum reduction pipelined per gate block.

kernel(**inputs) takes the full unsharded inputs and returns the full
[1024, 1] output; sharding/unsharding happens on the host inside.
"""

import os

import numpy as np

import concourse.bass as bass
import concourse.mybir as mybir
import concourse.tile as tile
from concourse import bacc
from concourse.bass import ds, ts
from concourse.masks import make_identity

F32 = mybir.dt.float32
F32R = mybir.dt.float32r
U32 = mybir.dt.uint32
I32 = mybir.dt.int32

# Problem constants (hardcoded per contract)
B, N, F = 1024, 200000, 64
K = 32          # neighbors
C, H = 16, 128  # gate channels, hidden
CORES = 8
QPC = B // CORES  # 128 queries per core
P = 128

GRP = 4096                     # scan group width (2 PSUM tiles, 1 SBUF tile)
PSW = 2048                     # PSUM tile width
NEG = -3.0e38                  # "minus inf" for match_replace


def build_program(n_pad=None, n_groups=None, loop_reps=1):
    """Build the per-core Bass program.

    loop_reps > 1 repeats the phase-1 scan loop (identical results) — used
    only for amortized hardware timing."""
    if n_groups is None:
        n_groups = (N + GRP - 1) // GRP          # 25
    if n_pad is None:
        n_pad = n_groups * GRP                   # 204800
    NG = n_groups
    NCAND = NG * 8                               # 200
    KF = F + 1    # 65 contraction (features + norm row)
    FW = F + 2    # 66 gather row width ([X | y | 0])

    SCAN_DT = F32 if os.environ.get("SCAN_FP32") else F32R
    GATE_DT = F32R if os.environ.get("GATE_F32R") else F32
    FOLD = int(os.environ.get("FOLD", "2048"))   # folded width per group
    assert GRP % FOLD == 0 and (GRP // FOLD) in (2, 4, 8)

    nc = bacc.Bacc(
        "TRN2",
        target_bir_lowering=False,
        debug=False,
        enable_asserts=False,
        num_devices=CORES,
    )

    xT = nc.dram_tensor("xT", [KF, QPC], SCAN_DT, kind="ExternalInput").ap()
    XtA = nc.dram_tensor("XtA", [KF, n_pad], SCAN_DT, kind="ExternalInput").ap()
    Xrow = nc.dram_tensor("Xrow", [n_pad, FW], F32, kind="ExternalInput").ap()
    Wg = nc.dram_tensor("Wg", [FW, C], GATE_DT, kind="ExternalInput").ap()
    W1 = nc.dram_tensor("W1", [F + C, H], F32, kind="ExternalInput").ap()
    Wl = nc.dram_tensor("Wl", [H, 1], F32, kind="ExternalInput").ap()
    bg = nc.dram_tensor("bg", [C, 1], F32, kind="ExternalInput").ap()
    b1 = nc.dram_tensor("b1", [H, 1], F32, kind="ExternalInput").ap()
    bl = nc.dram_tensor("bl", [1, 1], F32, kind="ExternalInput").ap()

    out = nc.dram_tensor("out", [1, QPC], F32, kind="ExternalOutput").ap()
    oidx = nc.dram_tensor("oidx", [QPC, K], F32, kind="ExternalOutput").ap()

    with tile.TileContext(nc) as tc:
        with tc.tile_pool(name="const", bufs=1) as const:
            xT_t = const.tile([KF, QPC], SCAN_DT)
            nc.sync.dma_start(xT_t[:], xT)
            Wg_t = const.tile([FW, C], GATE_DT)
            nc.sync.dma_start(Wg_t[:], Wg)
            W1_t = const.tile([F + C, H], F32)
            nc.sync.dma_start(W1_t[:], W1)
            Wl_t = const.tile([H, 1], F32)
            nc.sync.dma_start(Wl_t[:], Wl)
            bg_t = const.tile([C, 1], F32)
            nc.sync.dma_start(bg_t[:], bg)
            b1_t = const.tile([H, 1], F32)
            nc.sync.dma_start(b1_t[:], b1)
            bl_t = const.tile([1, 1], F32)
            nc.sync.dma_start(bl_t[:], bl)
            ident = const.tile([P, P], F32)
            make_identity(nc, ident[:])

            iota_u = const.tile([P, NCAND], U32)
            nc.gpsimd.iota(iota_u[:], pattern=[[1, NCAND]], base=0,
                           channel_multiplier=0)
            iota_f = const.tile([P, NCAND], F32)
            nc.vector.tensor_copy(iota_f[:], iota_u[:])
            # base[c] = (c // 8) * GRP — scan-group base of candidate column
            base_u = const.tile([P, NCAND], U32)
            nc.gpsimd.iota(base_u[:], pattern=[[GRP, NG], [0, 8]], base=0,
                           channel_multiplier=0)

            cand_val = const.tile([P, NCAND], F32)
            cand_pos = const.tile([P, NCAND], U32)
            cand_gidx = const.tile([P, NCAND], F32)
            stt_d = const.tile([P, NCAND], F32)   # DVE extraction scratch
            stt_p = const.tile([P, NCAND], F32)   # Pool extraction scratch

            # ---- phase 1: stream scores; fold + top-8 + exact positions ----
            # Per 8192-group: 4 PSUM tiles are produced by the PE and drained
            # to one SBUF tile by ACT; the Pool engine folds pairs of PSUM
            # tiles (g1), then halves (g2, g3); DVE takes top-8 of the folded
            # g3 and recovers exact in-group positions with one max_index
            # scan of the full SBUF copy.
            _rhsb = int(os.environ.get("RHS_BUFS", "3"))
            _scb = int(os.environ.get("SC_BUFS", "4"))
            with (
                tc.tile_pool(name="rhs", bufs=_rhsb) as rhsp,
                tc.tile_pool(name="sc", bufs=_scb) as scp,
                tc.tile_pool(name="g1", bufs=1) as g1p,
                tc.tile_pool(name="g2", bufs=2) as g2p,
                tc.tile_pool(name="psc", bufs=2, space="PSUM") as psc,
            ):
                _abl = os.environ.get("ABL", "")
                for s in [i for _ in range(loop_reps) for i in range(NG)]:
                    scg = scp.tile([P, GRP], F32)
                    for h in range(2):
                        rhs = rhsp.tile([KF, PSW], SCAN_DT)
                        eng = nc.sync if h % 2 == 0 else nc.scalar
                        eng.dma_start(rhs[:], XtA[:, ds(s * GRP + h * PSW, PSW)])
                        ps = psc.tile([P, PSW], F32)
                        for j0 in range(0, PSW, 512):
                            nc.tensor.matmul(
                                ps[:, ds(j0, 512)],
                                lhsT=xT_t[:],
                                rhs=rhs[:, ds(j0, 512)],
                                start=True, stop=True,
                            )
                        nc.scalar.copy(scg[:, ds(h * PSW, PSW)], ps[:])
                    # For a tunable fraction of groups, fold 4096 -> 2048 on
                    # the DMA engines (copy of low half on the HW queues +
                    # one max-combine accum-DMA on the SWDGE queue) to halve
                    # the DVE max8 scan; the rest run max8 on the full group.
                    _ffn, _ffd = (os.environ.get("FOLD_FRAC", "0/1") + "/1") \
                        .split("/")[:2]
                    use_fold = (s % int(_ffd)) < int(_ffn)
                    if use_fold:
                        g1 = g2p.tile([P, FOLD], F32, tag="gfold")
                        (nc.sync if s % 2 == 0 else nc.scalar).dma_start(
                            g1[:], scg[:, ds(0, FOLD)]
                        )
                        nc.gpsimd.dma_start(
                            g1[:], scg[:, ds(FOLD, FOLD)],
                            accum_op=mybir.AluOpType.max,
                        )
                        f_prev = g1
                    else:
                        f_prev = scg
                    # top-8 of folded values; exact positions from group scan
                    if _abl != "nomax":
                        nc.vector.max(cand_val[:, ts(s, 8)], f_prev[:])
                        if _abl != "nomi":
                            nc.vector.max_index(
                                cand_pos[:, ts(s, 8)], cand_val[:, ts(s, 8)],
                                scg[:]
                            )

            # global candidate index = scan-group base + within-group pos
            nc.vector.tensor_tensor(
                cand_pos[:], cand_pos[:], base_u[:], op=mybir.AluOpType.add
            )
            nc.vector.tensor_copy(cand_gidx[:], cand_pos[:])  # u32 -> f32

            # ---- phases 2-4 interleaved: merge -> extract -> gather ----
            wval = const.tile([P, K], F32)
            wpos = const.tile([P, K], U32)
            wposf = const.tile([P, K], F32)
            gidx = const.tile([P, K], F32)
            idx_i32 = const.tile([P, K], I32)
            nf = const.tile([P, K, FW], F32)
            nfT = const.tile([FW, K * P], GATE_DT)
            gatedT = const.tile([C, K * P], F32)
            with tc.tile_pool(name="psm", bufs=2, space="PSUM") as psm:
                for r in range(4):
                    nc.vector.max(wval[:, ts(r, 8)], cand_val[:])
                    nc.vector.max_index(
                        wpos[:, ts(r, 8)], wval[:, ts(r, 8)], cand_val[:]
                    )
                    if r < 3:
                        nc.vector.match_replace(
                            cand_val[:], wval[:, ts(r, 8)], cand_val[:],
                            imm_value=NEG,
                        )
                    nc.vector.tensor_copy(
                        wposf[:, ts(r, 8)], wpos[:, ts(r, 8)]
                    )  # u32 -> f32
                    # winner-index extraction (DVE STT), then per-winner
                    # gather + transpose so round r's gathers overlap round
                    # r+1's merge ops
                    for k in range(r * 8, r * 8 + 8):
                        nc.vector.scalar_tensor_tensor(
                            out=stt_d[:],
                            in0=iota_f[:],
                            scalar=wposf[:, k : k + 1],
                            in1=cand_gidx[:],
                            op0=mybir.AluOpType.is_equal,
                            op1=mybir.AluOpType.mult,
                            accum_out=gidx[:, k : k + 1],
                        )
                        nc.vector.tensor_copy(
                            idx_i32[:, k : k + 1], gidx[:, k : k + 1]
                        )
                        nc.gpsimd.indirect_dma_start(
                            out=nf[:, k, :],
                            out_offset=None,
                            in_=Xrow,
                            in_offset=bass.IndirectOffsetOnAxis(
                                ap=idx_i32[:, k : k + 1], axis=0
                            ),
                        )
                        pt = psm.tile([FW, P], F32, tag="pt")
                        nc.tensor.transpose(pt[:], nf[:, k, :], ident[:])
                        nc.scalar.copy(nfT[:, ts(k, P)], pt[:])
                nc.sync.dma_start(oidx, gidx[:])

                # ---- phase 5: gate MLP head ----
                # per 512-block: matmul + tanh, then a pipelined partial
                # reduce over the block's 4 neighbor-columns on DVE
                aggT = const.tile([C, P], F32)
                part = const.tile([C, P], F32)
                NBLK = (K * P) // 512
                for j in range(NBLK):
                    gp = psm.tile([C, 512], F32, tag="gp")
                    nc.tensor.matmul(
                        gp[:],
                        lhsT=Wg_t[:],
                        rhs=nfT[:, ts(j, 512)],
                        start=True,
                        stop=True,
                    )
                    nc.scalar.activation(
                        gatedT[:, ts(j, 512)],
                        gp[:],
                        mybir.ActivationFunctionType.Tanh,
                        bias=bg_t[:],
                    )
                    blk = gatedT[:, ts(j, 512)].rearrange(
                        "c (k q) -> c q k", k=4
                    )
                    if j == 0:
                        nc.vector.reduce_sum(
                            aggT[:], blk, axis=mybir.AxisListType.X
                        )
                    else:
                        nc.vector.reduce_sum(
                            part[:], blk, axis=mybir.AxisListType.X
                        )
                        nc.vector.tensor_tensor(
                            aggT[:], aggT[:], part[:],
                            op=mybir.AluOpType.add,
                        )

                oc = const.tile([F + C, P], F32)
                nc.vector.tensor_copy(oc[0:F, :], xT_t[0:F, :].bitcast(F32))
                nc.vector.tensor_copy(oc[F : F + C, :], aggT[:])

                h1p = psm.tile([H, P], F32, tag="h1p")
                nc.tensor.matmul(h1p[:], lhsT=W1_t[:], rhs=oc[:], start=True, stop=True)
                h1 = const.tile([H, P], F32)
                nc.scalar.activation(
                    h1[:], h1p[:], mybir.ActivationFunctionType.Tanh, bias=b1_t[:]
                )

                op_ = psm.tile([1, P], F32, tag="op")
                nc.tensor.matmul(op_[:], lhsT=Wl_t[:], rhs=h1[:], start=True, stop=True)
                outt = const.tile([1, P], F32)
                nc.scalar.activation(
                    outt[:], op_[:], mybir.ActivationFunctionType.Sigmoid, bias=bl_t[:]
                )
                nc.sync.dma_start(out, outt[:])

    nc.compile()
    return nc


def prep_inputs(x, X_data, y, W_gate, b_gate, W1, b1, W_last, b_last,
                n_pad=None, n_groups=None):
    """Host-side marshalling: build per-core input maps."""
    if n_groups is None:
        n_groups = (len(X_data) + GRP - 1) // GRP
    if n_pad is None:
        n_pad = n_groups * GRP
    n = len(X_data)
    KF = F + 1
    FW = F + 2

    x = np.asarray(x, np.float32)
    X_data = np.asarray(X_data, np.float32)
    y = np.asarray(y, np.float32)

    XtA = np.zeros((KF, n_pad), np.float32)
    XtA[:F, :n] = X_data.T
    XtA[F, :n] = -0.5 * (X_data * X_data).sum(1)  # fp32, as the reference computes
    XtA[F, n:] = -1.0e30

    Xrow = np.zeros((n_pad, FW), np.float32)
    Xrow[:n, :F] = X_data
    Xrow[:n, F] = y

    Wg = np.zeros((FW, C), np.float32)
    Wg[: F + 1] = np.asarray(W_gate, np.float32)

    shared = {
        "XtA": XtA,
        "Xrow": Xrow,
        "Wg": Wg,
        "W1": np.asarray(W1, np.float32),
        "Wl": np.asarray(W_last, np.float32).reshape(H, 1),
        "bg": np.asarray(b_gate, np.float32).reshape(C, 1),
        "b1": np.asarray(b1, np.float32).reshape(H, 1),
        "bl": np.asarray(b_last, np.float32).reshape(1, 1),
    }
    in_maps = []
    for c in range(CORES):
        xc = x[c * QPC : (c + 1) * QPC]
        xTa = np.ones((KF, QPC), np.float32)
        xTa[:F] = xc.T
        m = dict(shared)
        m["xT"] = xTa
        in_maps.append(m)
    return in_maps


_NC_CACHE = {}


def _get_program():
    if "nc" not in _NC_CACHE:
        _NC_CACHE["nc"] = build_program()
    return _NC_CACHE["nc"]


def kernel(x, X_data, y, W_gate, b_gate, W1, b1, W_last, b_last):
    from concourse import bass_utils

    nc = _get_program()
    in_maps = prep_inputs(x, X_data, y, W_gate, b_gate, W1, b1, W_last, b_last)
    res = bass_utils.run_bass_kernel_spmd(
        nc, in_maps, core_ids=list(range(CORES))
    )
    outs = [res.results[c]["out"].reshape(QPC) for c in range(CORES)]
    return np.concatenate(outs).reshape(B, 1).astype(np.float32)
